# revision 4
# baseline (speedup 1.0000x reference)
"""Trainium2 Bass kernel for Ac4kAttentionOp (int8 q/k + fp8e4m3 v quantized attention).

Shapes: q,k,v [B=2, H=16, N=2048, D=64] fp32 -> out [2,16,2048,64] fp32.
Sharding: 32 (B,H) heads split 4-per-core across 8 NeuronCores; no collectives.

Math (mirrors the reference exactly up to fp32 rounding order):
  k <- k - mean_N(k)
  qq = round(q / sf_q), sf_q = max(amax_D(q)/127, eps)      (per token)
  kq = round(k / sf_k), sf_k = max(amax_D(k)/127, eps)      (per token)
  vq = fp8e4m3(v / sf_v), sf_v = max(amax_N(v)/(448/2.25), eps)  (per channel)
  s^T[m,nq] = sum_d kq[m,d] * (qq[nq,d]*sf_q[nq]*sm) ;  p^T = exp(sf_k[m] * s^T)
  outT[d,nq] = sum_m vq[m,d] * p^T[m,nq] ; denom = ones-column of vq_aug
  out[nq,d] = outT[d,nq] * sf_v[d] / denom[nq]

Performance structure (why it looks like this):
  - All main-loop matmuls (QK and PV) use 128-row stationaries: kqT/qcsT are
    zero-padded from 64 to 128 contraction rows.  Row-count changes between
    consecutive matmuls serialize LDWEIGHTS (no weight-preload overlap) and
    the resulting per-matmul bubbles keep the PE at its 1.2GHz mid p-state;
    with uniform 128-row shapes the PE ramps to 2.4GHz and stays there.
  - Lookahead emission: QK(mt+2) is emitted before PV(mt) so the in-order PE
    queue always has runnable work while ACT computes exp(mt).
  - The ACT engine (exp over all N^2 scores) is the throughput floor
    (~133us busy per core); per-head prep/epilogue PE+DVE work is placed in
    the emission stream so ACT is never starved of QK outputs.
"""
import math
from contextlib import ExitStack

import numpy as np

import concourse.bass as bass
import concourse.tile as tile
from concourse import mybir
from concourse.masks import make_identity

B, H, N, D = 2, 16, 2048, 64
NCORES = 8
HEADS_PER_CORE = (B * H) // NCORES          # 4
SM_SCALE = 1.0 / math.sqrt(D)               # 0.125 (exact power of 2)
MAGIC = 12582912.0                          # 1.5*2^23: fp32 RNE integer round
INT8_MAX = 127.0
F8_AMAX_DIV = float(np.float32(448.0) / np.float32(2.25))  # FP8_MAX / MAX_SCALE
EPS = 1e-8

f32 = mybir.dt.float32
bf16 = mybir.dt.bfloat16
f16 = mybir.dt.float16
f8e4 = mybir.dt.float8e4
ALU = mybir.AluOpType
ACTF = mybir.ActivationFunctionType


def _bc(t: bass.AP, dims, off: int = 0) -> bass.AP:
    """Build a broadcast/restrided view of a tile AP (off in elements)."""
    return bass.AP(tensor=t.tensor, offset=t.offset + off, ap=dims)


def build_attention(nc: bass.Bass, heads: int = HEADS_PER_CORE, n: int = N,
                    bench_loops: int = 0):
    T = n // 128          # token tiles per head
    C = T // 2            # 128-wide transpose chunks
    NQH = n // 2          # query-half width (PSUM budget)
    q_d = nc.dram_tensor("q", [heads, n, D], f32, kind="ExternalInput").ap()
    k_d = nc.dram_tensor("k", [heads, n, D], f32, kind="ExternalInput").ap()
    v_d = nc.dram_tensor("v", [heads, n, D], f32, kind="ExternalInput").ap()
    o_d = nc.dram_tensor("out", [heads, n, D], f32, kind="ExternalOutput").ap()

    with tile.TileContext(nc) as tc, ExitStack() as ctx:
        singles = ctx.enter_context(tc.tile_pool(name="singles", bufs=1))
        loads = ctx.enter_context(tc.tile_pool(name="loads", bufs=2))
        work = ctx.enter_context(tc.tile_pool(name="work", bufs=2))
        scales = ctx.enter_context(tc.tile_pool(name="scales", bufs=2))
        small = ctx.enter_context(tc.tile_pool(name="small", bufs=4))
        opnds = ctx.enter_context(tc.tile_pool(name="opnds", bufs=2))
        pbuf = ctx.enter_context(tc.tile_pool(name="pbuf", bufs=4))
        obuf = ctx.enter_context(tc.tile_pool(name="obuf", bufs=2))
        ostore = ctx.enter_context(tc.tile_pool(name="ostore", bufs=4))
        osb = ctx.enter_context(tc.tile_pool(name="osb", bufs=2))
        ps_s = ctx.enter_context(tc.tile_pool(name="ps_s", bufs=2, space="PSUM"))
        ps_o = ctx.enter_context(tc.tile_pool(name="ps_o", bufs=1, space="PSUM"))
        ps_t = ctx.enter_context(tc.tile_pool(name="ps_t", bufs=2, space="PSUM"))

        ident_f = singles.tile([128, 128], f32)
        make_identity(nc, ident_f)
        ident_h = singles.tile([128, 128], f16)
        make_identity(nc, ident_h)
        ones_row = singles.tile([1, 128], f32)
        nc.gpsimd.memset(ones_row, 1.0)
        # constant [128,128] of 1/n in f16 (2^-11, exact): k-mean matmul weights
        invn_h = singles.tile([128, 128], f16)
        nc.gpsimd.memset(invn_h, 1.0 / n)

        if bench_loops:
            ctx.enter_context(tc.For_i(0, bench_loops, 1))

        # warm the ACT exp table before the first real exp
        warm = singles.tile([1, 1], f32)
        nc.gpsimd.memset(warm, 0.0)
        nc.scalar.activation(warm, warm, ACTF.Exp)

        def quant_int8(x_sb, tagpfx):
            """per-token int8 quantize: returns (q_rounded_f32, sf [128,T])."""
            amax = scales.tile([128, T], f32, tag=tagpfx + "amax")
            nc.vector.tensor_reduce(out=amax, in_=x_sb,
                                    axis=mybir.AxisListType.X, op=ALU.max,
                                    apply_absolute_value=True)
            sf = scales.tile([128, T], f32, tag=tagpfx + "sf")
            nc.vector.tensor_scalar(out=sf, in0=amax,
                                    scalar1=1.0 / INT8_MAX, scalar2=EPS,
                                    op0=ALU.mult, op1=ALU.max)
            rsf = scales.tile([128, T], f32, tag=tagpfx + "rsf")
            nc.vector.reciprocal(rsf, sf)
            xq = work.tile([128, T, D], f32, tag=tagpfx + "xq")
            nc.vector.tensor_mul(xq, x_sb,
                                 _bc(rsf, [rsf.ap[0], [1, T], [0, D]]))
            # RNE integer round: (x + MAGIC) - MAGIC
            nc.vector.tensor_scalar(out=xq, in0=xq,
                                    scalar1=MAGIC, scalar2=MAGIC,
                                    op0=ALU.add, op1=ALU.subtract)
            return xq, sf

        def load(h):
            q_sb = loads.tile([128, T, D], f32, tag="q_sb")
            nc.sync.dma_start(out=q_sb,
                              in_=q_d[h].rearrange("(t p) d -> p t d", p=128))
            k_sb = loads.tile([128, T, D], f32, tag="k_sb")
            nc.sync.dma_start(out=k_sb,
                              in_=k_d[h].rearrange("(t p) d -> p t d", p=128))
            v_sb = loads.tile([128, T, D], f32, tag="v_sb")
            nc.sync.dma_start(out=v_sb,
                              in_=v_d[h].rearrange("(t p) d -> p t d", p=128))
            return q_sb, k_sb, v_sb

        def prep_mean(bufs):
            """k mean via one f16 matmul + DVE reduce (PE + DVE, short)."""
            q_sb, k_sb, v_sb = bufs
            k_h = work.tile([128, T, D], f16, tag="k_h")
            nc.vector.tensor_copy(k_h, k_sb)
            mean_ps = ps_s.tile([128, NQH], f32, tag="pss")
            half_td = T * D // 2
            nc.tensor.matmul(mean_ps[:, 0:half_td], invn_h,
                             k_h[:, 0:T // 2, :], start=True, stop=True)
            nc.tensor.matmul(mean_ps[:, half_td:T * D], invn_h,
                             k_h[:, T // 2:T, :], start=True, stop=True)
            # reduce over t: view free dims as [D outer stride 1, T inner stride D]
            meanb = small.tile([128, D], f32, tag="meanb")
            nc.vector.tensor_reduce(
                out=meanb,
                in_=_bc(mean_ps, [mean_ps.ap[0], [1, D], [D, T]]),
                axis=mybir.AxisListType.X, op=ALU.add)
            return meanb

        def prep_quant(bufs, meanb):
            """DVE quant chain for q/k/v (no PE work)."""
            st = {}
            q_sb, k_sb, v_sb = bufs

            ks = work.tile([128, T, D], f32, tag="ks")
            nc.vector.tensor_sub(ks, k_sb,
                                 _bc(meanb, [meanb.ap[0], [0, T], [1, D]]))
            kq, sf_k = quant_int8(ks, "k")
            kq_h = work.tile([128, T, D], f16, tag="kq_h")
            nc.vector.tensor_copy(kq_h, kq)

            qq, sf_q = quant_int8(q_sb, "q")
            csfq = scales.tile([128, T], f32, tag="csfq")
            nc.vector.tensor_scalar_mul(csfq, sf_q, SM_SCALE)
            qcs = work.tile([128, T, D], f32, tag="qcs")
            nc.vector.tensor_mul(qcs, qq,
                                 _bc(csfq, [csfq.ap[0], [1, T], [0, D]]))
            qcs_h = work.tile([128, T, D], f16, tag="qcs_h")
            nc.vector.tensor_copy(qcs_h, qcs)

            # v: per-channel amax over tokens (partition dim via later PE transpose)
            amax_vp = work.tile([128, D], f32, tag="amax_vp")
            nc.vector.tensor_reduce(
                out=amax_vp,
                in_=_bc(v_sb, [v_sb.ap[0], [1, D], [D, T]]),
                axis=mybir.AxisListType.X, op=ALU.max,
                apply_absolute_value=True)

            st.update(kq_h=kq_h, qcs_h=qcs_h, sf_k=sf_k, amax_vp=amax_vp,
                      v_sb=v_sb)
            return st

        def transpose_split(x_h, dstT, tag):
            """[128,(T,64)] fp16 -> top half of padded [128,(T,128)] via PE
            chunk transposes (parity-stacked), then two strided parity-split
            DMAs. dstT partitions 64..127 are zeroed separately."""
            stk = work.tile([128, C, 128], f16, tag=tag + "_st")
            for c in range(C):
                tp = ps_t.tile([128, 128], f16, tag="pst")
                nc.tensor.transpose(tp, x_h[:, 2 * c:2 * c + 2, :], ident_h)
                nc.vector.tensor_copy(stk[:, c, :], tp)
            d64 = dstT[0:64]
            nc.sync.dma_start(
                out=_bc(d64, [d64.ap[0], [2 * 128, C], [1, 128]]),
                in_=stk[0:64, :, :])
            nc.sync.dma_start(
                out=_bc(d64, [d64.ap[0], [2 * 128, C], [1, 128]], off=128),
                in_=stk[64:128, :, :])

        def prep_pe(st):
            """PE transposes into padded operands + v-scale finish."""
            kqT = opnds.tile([128, T, 128], f16, tag="kqT")
            nc.gpsimd.memset(kqT[64:128, :, :], 0.0)
            transpose_split(st["kq_h"], kqT, "kqT")
            qcsT = opnds.tile([128, T, 128], f16, tag="qcsT")
            nc.gpsimd.memset(qcsT[64:128, :, :], 0.0)
            transpose_split(st["qcs_h"], qcsT, "qcsT")

            # v scales: transpose amax to channel-major, sf/rsf, broadcast rsf
            vt_ps = ps_t.tile([D, 128], f32, tag="pst")
            nc.tensor.transpose(vt_ps, st["amax_vp"], ident_f)
            amax_vT = scales.tile([D, 1], f32, tag="amax_vT")
            nc.vector.tensor_reduce(out=amax_vT, in_=vt_ps,
                                    axis=mybir.AxisListType.X, op=ALU.max)
            sf_vT = scales.tile([D, 1], f32, tag="sf_vT")
            nc.vector.tensor_scalar(out=sf_vT, in0=amax_vT,
                                    scalar1=1.0 / F8_AMAX_DIV, scalar2=EPS,
                                    op0=ALU.mult, op1=ALU.max)
            rsf_vT = scales.tile([D, 1], f32, tag="rsf_vT")
            nc.vector.reciprocal(rsf_vT, sf_vT)
            sfv65 = scales.tile([65, 1], f32, tag="sfv65")
            nc.gpsimd.memset(sfv65, 1.0)
            nc.vector.tensor_copy(sfv65[0:D, :], sf_vT)
            rsf_row = small.tile([1, D], f32, tag="rsf_row")
            nc.sync.dma_start(out=rsf_row, in_=rsf_vT)
            rsf_bps = ps_t.tile([128, D], f32, tag="pst")
            nc.tensor.matmul(rsf_bps, ones_row, rsf_row, start=True, stop=True)
            rsf_b = small.tile([128, D], f32, tag="rsf_b")
            nc.vector.tensor_copy(rsf_b, rsf_bps)
            vq_pre = work.tile([128, T, D], f32, tag="vq_pre")
            nc.vector.tensor_mul(vq_pre, st["v_sb"],
                                 _bc(rsf_b, [rsf_b.ap[0], [0, T], [1, D]]))
            vq_f8 = work.tile([128, T, D], f8e4, tag="vq_f8")
            nc.vector.tensor_copy(vq_f8, vq_pre)
            vq_aug = opnds.tile([128, T, D + 1], f16, tag="vq_aug")
            nc.vector.tensor_copy(vq_aug[:, :, 0:D], vq_f8)
            nc.gpsimd.memset(vq_aug[:, :, D:D + 1], 1.0)

            return dict(kqT=kqT, qcsT=qcsT, vq_aug=vq_aug, sf_k=st["sf_k"],
                        sfv65=sfv65)

        def half_loop(h, st, half):
            """Main QK->exp->PV loop for one query half (NQH queries).
            Lookahead emission: QK(mt+2)/exp(mt+2) before PV(mt)."""
            kqT, qcsT, vq_aug = st["kqT"], st["qcsT"], st["vq_aug"]
            sf_k = st["sf_k"]
            TH = T // 2

            def qk_exp(mt):
                s_ps = ps_s.tile([128, NQH], f32, tag="pss")
                for j in range(NQH // 512):
                    rhs = qcsT[:, half * TH + 4 * j:half * TH + 4 * (j + 1), :]
                    nc.tensor.matmul(s_ps[:, j * 512:(j + 1) * 512],
                                     kqT[:, mt, :], rhs, start=True, stop=True)
                p_sb = pbuf.tile([128, NQH], f16, tag="p_sb")
                nc.scalar.activation(p_sb, s_ps, ACTF.Exp,
                                     scale=sf_k[:, mt:mt + 1])
                return p_sb

            o_ps = ps_o.tile([65, NQH], f32, tag="pso")
            ps = [qk_exp(0), qk_exp(1)]
            for mt in range(T):
                if mt + 2 < T:
                    ps.append(qk_exp(mt + 2))
                p_sb = ps[mt]
                for j in range(NQH // 512):
                    nc.tensor.matmul(
                        o_ps[:, j * 512:(j + 1) * 512],
                        vq_aug[:, mt, :],
                        p_sb[:, j * 512:(j + 1) * 512],
                        start=(mt == 0), stop=(mt == T - 1))
            # scale by per-channel v scale, park in SBUF (frees the psum bank)
            outT_sb = obuf.tile([65, NQH], f32, tag="outT")
            nc.vector.tensor_scalar_mul(outT_sb, o_ps, st["sfv65"][:, 0:1])
            return outT_sb

        def head_epilogue(h, outTs):
            """Grouped out-transposes + denominator divide + single store."""
            out_sb = osb.tile([128, T, D], f32, tag="out_sb")
            for half in range(2):
                outT_sb = outTs[half]
                for c in range(NQH // 128):
                    tp2 = ps_t.tile([128, 65], f32, tag="pst")
                    nc.tensor.transpose(tp2, outT_sb[:, c * 128:(c + 1) * 128],
                                        ident_f[0:65, 0:65])
                    rec = ostore.tile([128, 1], f32, tag="rec")
                    nc.vector.reciprocal(rec, tp2[:, D:D + 1])
                    nc.vector.tensor_mul(
                        out_sb[:, half * (T // 2) + c, :], tp2[:, 0:D],
                        _bc(rec, [rec.ap[0], [0, D]]))
            nc.sync.dma_start(
                out=o_d[h].rearrange("(t p) d -> p t d", p=128),
                in_=out_sb)

        # ---- head pipeline ----
        # Emission per head h (engines run in emission order, per engine):
        #   loads(h+1); cast/mean feed emitted mid-head; quant DVE chain after
        #   half0; prep PE transposes after half1; then head epilogue.
        bufs = load(0)
        meanb = prep_mean(bufs)
        st_q = prep_quant(bufs, meanb)
        st = prep_pe(st_q)
        bufs_next = None
        for h in range(heads):
            if h + 1 < heads:
                bufs_next = load(h + 1)
            outT0 = half_loop(h, st, 0)
            if h + 1 < heads:
                meanb_n = prep_mean(bufs_next)
                st_qn = prep_quant(bufs_next, meanb_n)
            outT1 = half_loop(h, st, 1)
            if h + 1 < heads:
                st_next = prep_pe(st_qn)
            head_epilogue(h, (outT0, outT1))
            if h + 1 < heads:
                st = st_next
    return nc


_CACHED = {}


def _get_nc():
    if "nc" not in _CACHED:
        from concourse import bacc

        nc = bacc.Bacc("TRN2", target_bir_lowering=False, debug=False)
        build_attention(nc)
        nc.compile()
        _CACHED["nc"] = nc
    return _CACHED["nc"]


def kernel(q: np.ndarray, k: np.ndarray, v: np.ndarray) -> np.ndarray:
    from concourse.bass_utils import run_bass_kernel_spmd

    nc = _get_nc()
    qf = np.ascontiguousarray(np.asarray(q, dtype=np.float32).reshape(B * H, N, D))
    kf = np.ascontiguousarray(np.asarray(k, dtype=np.float32).reshape(B * H, N, D))
    vf = np.ascontiguousarray(np.asarray(v, dtype=np.float32).reshape(B * H, N, D))
    hpc = HEADS_PER_CORE
    in_maps = [
        {"q": qf[c * hpc:(c + 1) * hpc],
         "k": kf[c * hpc:(c + 1) * hpc],
         "v": vf[c * hpc:(c + 1) * hpc]}
        for c in range(NCORES)
    ]
    res = run_bass_kernel_spmd(nc, in_maps, core_ids=list(range(NCORES)))
    out = np.concatenate([np.asarray(r["out"]) for r in res.results], axis=0)
    return out.reshape(B, H, N, D).astype(np.float32)
